# revision 20
# baseline (speedup 1.0000x reference)
"""Expert-parallel Trainium2 kernel for nn_BlockchainAIEngine (MoE + consensus MLP).

Math (reference):
    h[e]    = relu(x @ W1[e] + b1[e])            e in [0,100)   x:[2048,512]
    outs[e] = h[e] @ W2[e] + b2[e]
    concat  = outs transposed/reshaped to [B, E*128]  (expert-major features)
    cons    = relu(concat @ Wc1 + bc1) @ Wc2 + bc2
    out     = (cons @ We + be) @ Wd + bd

Device strategy (8 cores):
  - Pad E 100->104, 13 experts per core.  Each core holds xT (replicated)
    and its experts' W1/W2/Wc1 slices in SBUF, all fp32r.
  - concat @ Wc1 == sum_e outs[e] @ Wc1[e*128:(e+1)*128]  -> per-core partial
    [512, B] accumulated in PSUM over local experts; b2's contribution is
    data-independent and folded into an effective bc1 on the host.
  - Per 512-column batch tile: AllReduce the 1MB partial across cores
    (overlaps the next tile's compute), then each core redundantly computes
    the small consensus/encoder/decoder tail; core 0's output is returned.
  - Everything is computed transposed ([feature, batch]); the host
    transposes the final [128, 2048] back to [2048, 128].
"""
import numpy as np

import concourse.bacc as bacc
import concourse.mybir as mybir
import concourse.tile as tile
from concourse.bass_utils import run_bass_kernel_spmd

E, DIN, H, DOUT = 100, 512, 256, 128
B = 2048
F1 = 512                     # consensus hidden width
NCORES = 8
EPC = 13                     # experts per core (104 padded)
EPAD = NCORES * EPC
NT, NB = 4, 512              # batch tiles: 4 x 512
KC = DIN // 128              # 4 k-chunks for layer 1
HC = H // 128                # 2 h-chunks
FC = F1 // 128               # 4 consensus-feature chunks

F32 = mybir.dt.float32
F32R = mybir.dt.float32r
RELU = mybir.ActivationFunctionType.Relu
COPY = mybir.ActivationFunctionType.Copy
IDENT = mybir.ActivationFunctionType.Identity

_CACHE = {}


def _build():
    nc = bacc.Bacc("TRN2", target_bir_lowering=False, debug=False,
                   num_devices=NCORES)

    def din(name, shape, dt=F32R):
        return nc.dram_tensor(name, list(shape), dt, kind="ExternalInput").ap()

    xT = din("xT", (128, KC, B))                  # [kk, kc, b]
    w1 = din("w1", (EPC, 128, KC * HC, 128))      # [e, kk, kc*2+mc, m]
    w2 = din("w2", (EPC, 128, HC, 128))           # [e, hh, hc, o]
    wc1 = din("wc1", (EPC, 128, FC, 128))         # [e, dd, mc, m]
    wc2 = din("wc2", (128, FC, 128))              # [kk, kc, m]
    we = din("we", (128, 128))
    wd = din("wd", (128, 128))
    b1 = din("b1", (128, EPC * HC), F32)          # [hh, e*2+hc]
    bc1e = din("bc1e", (128, FC), F32)            # effective bc1 (b2 folded in)
    bc2 = din("bc2", (128, 1), F32)
    be = din("be", (128, 1), F32)
    bd = din("bd", (128, 1), F32)
    outT = nc.dram_tensor("outT", [128, B], F32, kind="ExternalOutput").ap()

    with tile.TileContext(nc) as tc:
        with (
            tc.tile_pool(name="wpool", bufs=1) as wpool,
            tc.tile_pool(name="act", bufs=2) as act,
            tc.tile_pool(name="ph", bufs=1, space="PSUM") as ph,
            tc.tile_pool(name="po", bufs=1, space="PSUM") as po,
            tc.tile_pool(name="pc", bufs=1, space="PSUM") as pc,
            tc.tile_pool(name="pt", bufs=1, space="PSUM") as pt,
            tc.tile_pool(name="dram", bufs=2, space="DRAM") as dram,
        ):
            # ---- resident tiles ----
            # small tensors FIRST: the first relu blocks on b1, and through
            # it the whole expert pipeline (hp slot reuse).
            b1t = wpool.tile([128, EPC * HC], F32, tag="b1")
            nc.sync.dma_start(b1t[:], b1[:])
            bc1et = wpool.tile([128, FC], F32, tag="bc1e")
            nc.sync.dma_start(bc1et[:], bc1e[:])
            bc2t = wpool.tile([128, 1], F32, tag="bc2")
            nc.sync.dma_start(bc2t[:], bc2[:])
            bet = wpool.tile([128, 1], F32, tag="be")
            nc.sync.dma_start(bet[:], be[:])
            bdt = wpool.tile([128, 1], F32, tag="bd")
            nc.sync.dma_start(bdt[:], bd[:])
            wc2t = wpool.tile([128, FC, 128], F32R, tag="wc2")
            nc.sync.dma_start(wc2t[:], wc2[:])
            wet = wpool.tile([128, 128], F32R, tag="we")
            nc.sync.dma_start(wet[:], we[:])
            wdt = wpool.tile([128, 128], F32R, tag="wd")
            nc.sync.dma_start(wdt[:], wd[:])

            # x (transposed). First batch tile split per k-chunk so it
            # spreads across DMA queues (single-queue BW is ~70 GB/s);
            # remaining tiles deferred so early expert weights land first.
            xt = wpool.tile([128, KC, B], F32R, tag="xt")
            for kc in range(KC):
                nc.sync.dma_start(xt[:, kc, 0:NB], xT[:, kc, 0:NB])

            w1t, w2t, wc1t = [], [], []
            for e in range(EPC):
                t1 = wpool.tile([128, KC * HC, 128], F32R, tag=f"w1_{e}")
                if e == 0:
                    for j in range(KC * HC):
                        nc.sync.dma_start(t1[:, j, :], w1[e][:, j, :])
                else:
                    nc.sync.dma_start(t1[:], w1[e])
                w1t.append(t1)
                t2 = wpool.tile([128, HC, 128], F32R, tag=f"w2_{e}")
                nc.sync.dma_start(t2[:], w2[e])
                w2t.append(t2)
                t3 = wpool.tile([128, FC, 128], F32R, tag=f"wc1_{e}")
                nc.sync.dma_start(t3[:], wc1[e])
                wc1t.append(t3)
                if e == 2:
                    for n in range(1, NT):
                        nc.sync.dma_start(xt[:, :, n * NB:(n + 1) * NB],
                                          xT[:, :, n * NB:(n + 1) * NB])

            # ---- main loop: flat (btile, expert) stream with a 2-deep ----
            # ---- software pipeline so the PE never waits on ACT/DVE  ----
            # stage A (idx):   L1 8mm -> hp, relu -> hsb
            # stage B (idx-1): L2 2mm -> op, DVE copy -> osb
            # stage C (idx-2): C1 4mm accumulate -> cons[btile]
            NIDX = NT * EPC
            # epochs between AllReduce issue and tail MMs; the first AR also
            # absorbs cross-core launch skew, so tail(0) waits much longer
            TAIL_DEFER = {0: 22, 1: 16, 2: 8}
            hs_q, op_q, osb_q, cons_q, tail_q = {}, {}, {}, {}, {}

            def stage_a(idx):
                n, e = divmod(idx, EPC)
                bs = slice(n * NB, (n + 1) * NB)
                if e == 0:
                    cons_q[n] = pc.tile([128, FC, NB], F32, tag="cons", name="cons")
                hs = []
                for mc in range(HC):
                    hp = ph.tile([128, NB], F32, tag=f"hp{mc}")
                    for kc in range(KC):
                        nc.tensor.matmul(
                            hp[:], w1t[e][:, kc * HC + mc, :], xt[:, kc, bs],
                            start=(kc == 0), stop=(kc == KC - 1),
                        )
                    hsb = act.tile([128, NB], F32R, tag=f"hsb{mc}")
                    nc.scalar.activation(
                        hsb[:], hp[:], RELU,
                        bias=b1t[:, e * HC + mc:e * HC + mc + 1])
                    hs.append(hsb)
                hs_q[idx] = hs

            def stage_b(idx):
                _, e = divmod(idx, EPC)
                hs = hs_q.pop(idx)
                op = po.tile([128, NB], F32, tag="op")
                for hc in range(HC):
                    nc.tensor.matmul(
                        op[:], w2t[e][:, hc, :], hs[hc][:],
                        start=(hc == 0), stop=(hc == HC - 1),
                    )
                osb = act.tile([128, NB], F32R, tag="osb")
                nc.vector.tensor_copy(osb[:], op[:])
                osb_q[idx] = osb

            def stage_c(idx):
                n, e = divmod(idx, EPC)
                osb = osb_q.pop(idx)
                cons = cons_q[n]
                for mc in range(FC):
                    nc.tensor.matmul(
                        cons[:, mc, :], wc1t[e][:, mc, :], osb[:],
                        start=(e == 0), stop=(e == EPC - 1),
                    )
                if e == EPC - 1:
                    finish_btile(n)

            def finish_btile(n):
                # PSUM -> SBUF -> DRAM per chunk (pipelines copy with DMA,
                # spreads DMA queues), then one 1MB AllReduce.  The ~21us
                # collective cost is mostly fixed, so never split it.
                cons = cons_q.pop(n)
                csb = act.tile([128, FC, NB], F32, tag="csb")
                cin = dram.tile([128, FC, NB], F32, tag="cin", name="cin")
                for mc in range(FC):
                    nc.vector.tensor_copy(csb[:, mc, :], cons[:, mc, :])
                    nc.sync.dma_start(cin[:, mc, :], csb[:, mc, :])
                cout = dram.tile([128, FC, NB], F32, tag="cout", name="cout")
                nc.gpsimd.collective_compute(
                    "AllReduce", mybir.AluOpType.add,
                    replica_groups=[list(range(NCORES))],
                    ins=[cin.opt()], outs=[cout.opt()],
                )
                tail_q[n] = cout

            def tail_btile(n, cout):
                # ---- tail (redundant on every core) ----
                bs = slice(n * NB, (n + 1) * NB)
                rsb = act.tile([128, FC, NB], F32, tag="rsb")
                rr = act.tile([128, FC, NB], F32R, tag="rr")
                c2p = pt.tile([128, NB], F32, tag="tailp")
                for fc in range(FC):
                    nc.sync.dma_start(rsb[:, fc, :], cout[:, fc, :])
                    nc.scalar.activation(rr[:, fc, :], rsb[:, fc, :],
                                         RELU, bias=bc1et[:, fc:fc + 1])
                    nc.tensor.matmul(c2p[:], wc2t[:, fc, :],
                                     rr[:, fc, :],
                                     start=(fc == 0), stop=(fc == FC - 1))
                c2s = act.tile([128, NB], F32R, tag="c2s")
                nc.scalar.activation(c2s[:], c2p[:], IDENT, bias=bc2t[:, 0:1])
                ep = pt.tile([128, NB], F32, tag="tailp")
                nc.tensor.matmul(ep[:], wet[:], c2s[:],
                                 start=True, stop=True)
                es = act.tile([128, NB], F32R, tag="es")
                nc.scalar.activation(es[:], ep[:], IDENT, bias=bet[:, 0:1])
                dp = pt.tile([128, NB], F32, tag="tailp")
                nc.tensor.matmul(dp[:], wdt[:], es[:],
                                 start=True, stop=True)
                ds = act.tile([128, NB], F32, tag="ds")
                nc.scalar.activation(ds[:], dp[:], IDENT, bias=bdt[:, 0:1])
                b0 = n * NB
                nc.sync.dma_start(outT[:, b0:b0 + NB // 2], ds[:, 0:NB // 2])
                nc.sync.dma_start(outT[:, b0 + NB // 2:b0 + NB],
                                  ds[:, NB // 2:NB])

            def flush_tails(idx):
                # emit tail(n) only once its AllReduce has had TAIL_DEFER
                # epochs (~27us) to complete, so tail MMs don't stall the PE
                for n in list(tail_q):
                    if idx is None or idx >= (n + 1) * EPC + 1 + TAIL_DEFER[n]:
                        tail_btile(n, tail_q.pop(n))

            for idx in range(NIDX):
                stage_a(idx)
                if idx >= 1:
                    stage_b(idx - 1)
                if idx >= 2:
                    stage_c(idx - 2)
                flush_tails(idx)
            stage_b(NIDX - 1)
            stage_c(NIDX - 2)
            stage_c(NIDX - 1)
            flush_tails(None)

    nc.compile()
    return nc


def _prep(x, W1, b1, W2, b2, Wc1, bc1, Wc2, bc2, We, be, Wd, bd):
    """Host-side reshape/pad of the full inputs into per-core device arrays."""
    f = np.float32
    c = np.ascontiguousarray

    # pad experts 100 -> 104 with zeros
    W1p = np.zeros((EPAD, DIN, H), f); W1p[:E] = W1
    W2p = np.zeros((EPAD, H, DOUT), f); W2p[:E] = W2
    b1p = np.zeros((EPAD, H), f); b1p[:E] = b1
    Wc1p = np.zeros((EPAD * DOUT, F1), f); Wc1p[:E * DOUT] = Wc1

    # fold b2 (and padded zeros) into an effective bc1
    bc1_eff = (bc1.astype(np.float64)
               + b2.astype(np.float64).ravel() @ Wc1.astype(np.float64)).astype(f)

    xT = c(x.T.reshape(KC, 128, B).transpose(1, 0, 2))            # [kk,kc,b]
    w1 = c(W1p.reshape(EPAD, KC, 128, HC, 128)
           .transpose(0, 2, 1, 3, 4).reshape(EPAD, 128, KC * HC, 128))
    w2 = c(W2p.reshape(EPAD, HC, 128, DOUT).transpose(0, 2, 1, 3))
    wc1 = c(Wc1p.reshape(EPAD, 128, FC, 128))
    b1h = c(b1p.reshape(EPAD, HC, 128).transpose(2, 0, 1)
            .reshape(128, EPAD * HC))                              # [hh, e*2+hc]
    wc2 = c(Wc2.reshape(FC, 128, DOUT).transpose(1, 0, 2))        # [kk,kc,m]
    bc1e = c(bc1_eff.reshape(FC, 128).T)                          # [ff,fc]

    shared = {
        "xT": xT, "wc2": wc2, "bc1e": bc1e,
        "we": c(We.astype(f)), "wd": c(Wd.astype(f)),
        "bc2": c(bc2.reshape(128, 1).astype(f)),
        "be": c(be.reshape(128, 1).astype(f)),
        "bd": c(bd.reshape(128, 1).astype(f)),
    }
    in_maps = []
    for core in range(NCORES):
        es = slice(core * EPC, (core + 1) * EPC)
        in_maps.append({
            **shared,
            "w1": c(w1[es]), "w2": c(w2[es]), "wc1": c(wc1[es]),
            "b1": c(b1h[:, core * EPC * HC:(core + 1) * EPC * HC]),
        })
    return in_maps


def kernel(x, W1, b1, W2, b2, Wc1, bc1, Wc2, bc2, We, be, Wd, bd,
           _trace=False):
    if "nc" not in _CACHE:
        _CACHE["nc"] = _build()
    nc = _CACHE["nc"]
    in_maps = _prep(x, W1, b1, W2, b2, Wc1, bc1, Wc2, bc2, We, be, Wd, bd)
    res = run_bass_kernel_spmd(nc, in_maps, list(range(NCORES)), trace=_trace)
    if _trace:
        _CACHE["last_result"] = res
    outT = res.results[0]["outT"]
    return np.ascontiguousarray(outT.T)


# revision 22
# speedup vs baseline: 1.0715x; 1.0715x over previous
"""Expert-parallel Trainium2 kernel for nn_BlockchainAIEngine (MoE + consensus MLP).

Math (reference):
    h[e]    = relu(x @ W1[e] + b1[e])            e in [0,100)   x:[2048,512]
    outs[e] = h[e] @ W2[e] + b2[e]
    concat  = outs transposed/reshaped to [B, E*128]  (expert-major features)
    cons    = relu(concat @ Wc1 + bc1) @ Wc2 + bc2
    out     = (cons @ We + be) @ Wd + bd

Device strategy (8 cores):
  - Pad E 100->104, 13 experts per core.  Each core holds xT (replicated)
    and its experts' W1/W2/Wc1 slices in SBUF, all fp32r.
  - concat @ Wc1 == sum_e outs[e] @ Wc1[e*128:(e+1)*128]  -> per-core partial
    [512, B] accumulated in PSUM over local experts; b2's contribution is
    data-independent and folded into an effective bc1 on the host.
  - Per 512-column batch tile: AllReduce the 1MB partial across cores
    (overlaps the next tile's compute), then each core redundantly computes
    the small consensus/encoder/decoder tail; core 0's output is returned.
  - Everything is computed transposed ([feature, batch]); the host
    transposes the final [128, 2048] back to [2048, 128].
"""
import numpy as np

import concourse.bacc as bacc
import concourse.mybir as mybir
import concourse.tile as tile
from concourse.bass_utils import run_bass_kernel_spmd

E, DIN, H, DOUT = 100, 512, 256, 128
B = 2048
F1 = 512                     # consensus hidden width
NCORES = 8
EPC = 13                     # experts per core (104 padded)
EPAD = NCORES * EPC
NT, NB = 4, 512              # batch tiles: 4 x 512
KC = DIN // 128              # 4 k-chunks for layer 1
HC = H // 128                # 2 h-chunks
FC = F1 // 128               # 4 consensus-feature chunks

F32 = mybir.dt.float32
F32R = mybir.dt.float32r
RELU = mybir.ActivationFunctionType.Relu
COPY = mybir.ActivationFunctionType.Copy
IDENT = mybir.ActivationFunctionType.Identity

_CACHE = {}


def _build():
    nc = bacc.Bacc("TRN2", target_bir_lowering=False, debug=False,
                   num_devices=NCORES)

    def din(name, shape, dt=F32R):
        return nc.dram_tensor(name, list(shape), dt, kind="ExternalInput").ap()

    xT = din("xT", (NT, 128, KC, NB))             # [n, kk, kc, b']
    w1 = din("w1", (EPC, 128, KC * HC, 128))      # [e, kk, kc*2+mc, m]
    wB = din("wB", (EPC, 128, HC + FC, 128))      # [e, *, w2(2) | wc1(4), *]
    wt = din("wt", (128, FC, 128))                # folded Wc2@We@Wd [kk, kc, m]
    # smalls: b1 (26) | bc1_eff (4) | btail (1)
    smalls = din("smalls", (128, EPC * HC + FC + 1), F32)
    outT = nc.dram_tensor("outT", [128, B], F32, kind="ExternalOutput").ap()

    with tile.TileContext(nc) as tc:
        with (
            tc.tile_pool(name="wpool", bufs=1) as wpool,
            tc.tile_pool(name="act", bufs=2) as act,
            tc.tile_pool(name="ph", bufs=1, space="PSUM") as ph,
            tc.tile_pool(name="po", bufs=1, space="PSUM") as po,
            tc.tile_pool(name="pc", bufs=1, space="PSUM") as pc,
            tc.tile_pool(name="pt", bufs=1, space="PSUM") as pt,
            tc.tile_pool(name="dram", bufs=2, space="DRAM") as dram,
        ):
            # ---- resident tiles ----
            # gating transfers first: x btile 0, then biases, then weights
            xt = wpool.tile([128, NT, KC, NB], F32R, tag="xt")
            nc.sync.dma_start(xt[:, 0], xT[0])
            smt = wpool.tile([128, EPC * HC + FC + 1], F32, tag="smt")
            nc.sync.dma_start(smt[:], smalls[:])
            wtt = wpool.tile([128, FC, 128], F32R, tag="wtt")
            nc.sync.dma_start(wtt[:], wt[:])

            w1t, wBt = [], []
            for e in range(EPC):
                t1 = wpool.tile([128, KC * HC, 128], F32R, tag=f"w1_{e}")
                nc.sync.dma_start(t1[:], w1[e])
                w1t.append(t1)
                tB = wpool.tile([128, HC + FC, 128], F32R, tag=f"wB_{e}")
                nc.sync.dma_start(tB[:], wB[e])
                wBt.append(tB)
                if e == 2:
                    for n in range(1, NT):
                        nc.sync.dma_start(xt[:, n], xT[n])

            # ---- main loop: flat (btile, expert) stream with a 2-deep ----
            # ---- software pipeline so the PE never waits on ACT/DVE  ----
            # stage A (idx):   L1 8mm -> hp, relu -> hsb
            # stage B (idx-1): L2 2mm -> op, DVE copy -> osb
            # stage C (idx-2): C1 4mm accumulate -> cons[btile]
            NIDX = NT * EPC
            # epochs between AllReduce issue and tail MMs; the first AR also
            # absorbs cross-core launch skew, so tail(0) waits much longer
            TAIL_DEFER = {0: 26, 1: 16, 2: 8}
            hs_q, op_q, osb_q, cons_q, tail_q = {}, {}, {}, {}, {}

            def stage_a(idx):
                n, e = divmod(idx, EPC)
                if e == 0:
                    cons_q[n] = pc.tile([128, FC, NB], F32, tag="cons", name="cons")
                hs = []
                for mc in range(HC):
                    hp = ph.tile([128, NB], F32, tag=f"hp{mc}")
                    for kc in range(KC):
                        nc.tensor.matmul(
                            hp[:], w1t[e][:, kc * HC + mc, :], xt[:, n, kc, :],
                            start=(kc == 0), stop=(kc == KC - 1),
                        )
                    hsb = act.tile([128, NB], F32R, tag=f"hsb{mc}")
                    nc.scalar.activation(
                        hsb[:], hp[:], RELU,
                        bias=smt[:, e * HC + mc:e * HC + mc + 1])
                    hs.append(hsb)
                hs_q[idx] = hs

            def stage_b(idx):
                _, e = divmod(idx, EPC)
                hs = hs_q.pop(idx)
                op = po.tile([128, NB], F32, tag="op")
                for hc in range(HC):
                    nc.tensor.matmul(
                        op[:], wBt[e][:, hc, :], hs[hc][:],
                        start=(hc == 0), stop=(hc == HC - 1),
                    )
                osb = act.tile([128, NB], F32R, tag="osb")
                nc.vector.tensor_copy(osb[:], op[:])
                osb_q[idx] = osb

            def stage_c(idx):
                n, e = divmod(idx, EPC)
                osb = osb_q.pop(idx)
                cons = cons_q[n]
                for mc in range(FC):
                    nc.tensor.matmul(
                        cons[:, mc, :], wBt[e][:, HC + mc, :], osb[:],
                        start=(e == 0), stop=(e == EPC - 1),
                    )
                if e == EPC - 1:
                    finish_btile(n)

            def finish_btile(n):
                # PSUM -> SBUF -> DRAM per chunk (pipelines copy with DMA,
                # spreads DMA queues), then one 1MB AllReduce.  The ~21us
                # collective cost is mostly fixed, so never split it.
                cons = cons_q.pop(n)
                csb = act.tile([128, FC, NB], F32, tag="csb")
                cin = dram.tile([128, FC, NB], F32, tag="cin", name="cin")
                for mc in range(FC):
                    nc.vector.tensor_copy(csb[:, mc, :], cons[:, mc, :])
                    nc.sync.dma_start(cin[:, mc, :], csb[:, mc, :])
                cout = dram.tile([128, FC, NB], F32, tag="cout", name="cout")
                nc.gpsimd.collective_compute(
                    "AllReduce", mybir.AluOpType.add,
                    replica_groups=[list(range(NCORES))],
                    ins=[cin.opt()], outs=[cout.opt()],
                )
                tail_q[n] = cout

            def tail_btile(n, cout):
                # ---- tail (redundant on every core) ----
                # out = relu(allred + bc1_eff) @ (Wc2 We Wd) + btail
                rsb = act.tile([128, FC, NB], F32, tag="rsb")
                rr = act.tile([128, FC, NB], F32R, tag="rr")
                c2p = pt.tile([128, NB], F32, tag="tailp")
                for fc in range(FC):
                    nc.sync.dma_start(rsb[:, fc, :], cout[:, fc, :])
                    nc.scalar.activation(
                        rr[:, fc, :], rsb[:, fc, :], RELU,
                        bias=smt[:, EPC * HC + fc:EPC * HC + fc + 1])
                    nc.tensor.matmul(c2p[:], wtt[:, fc, :], rr[:, fc, :],
                                     start=(fc == 0), stop=(fc == FC - 1))
                ds = act.tile([128, NB], F32, tag="ds")
                nc.scalar.activation(ds[:], c2p[:], IDENT,
                                     bias=smt[:, EPC * HC + FC:])
                nc.sync.dma_start(outT[:, n * NB:(n + 1) * NB], ds[:])

            def flush_tails(idx):
                # emit tail(n) only once its AllReduce has had TAIL_DEFER
                # epochs (~27us) to complete, so tail MMs don't stall the PE
                for n in list(tail_q):
                    if idx is None or idx >= (n + 1) * EPC + 1 + TAIL_DEFER[n]:
                        tail_btile(n, tail_q.pop(n))

            for idx in range(NIDX):
                stage_a(idx)
                if idx >= 1:
                    stage_b(idx - 1)
                if idx >= 2:
                    stage_c(idx - 2)
                flush_tails(idx)
            stage_b(NIDX - 1)
            stage_c(NIDX - 2)
            stage_c(NIDX - 1)
            flush_tails(None)

    nc.compile()
    return nc


def _prep(x, W1, b1, W2, b2, Wc1, bc1, Wc2, bc2, We, be, Wd, bd):
    """Host-side reshape/pad of the full inputs into per-core device arrays."""
    f = np.float32
    c = np.ascontiguousarray

    # pad experts 100 -> 104 with zeros
    W1p = np.zeros((EPAD, DIN, H), f); W1p[:E] = W1
    W2p = np.zeros((EPAD, H, DOUT), f); W2p[:E] = W2
    b1p = np.zeros((EPAD, H), f); b1p[:E] = b1
    Wc1p = np.zeros((EPAD * DOUT, F1), f); Wc1p[:E * DOUT] = Wc1

    # fold b2 (and padded zeros) into an effective bc1
    bc1_eff = (bc1.astype(np.float64)
               + b2.astype(np.float64).ravel() @ Wc1.astype(np.float64)).astype(f)

    # folded linear tail: Wtail = Wc2 @ We @ Wd,  btail = bc2@We@Wd + be@Wd + bd
    Wtail64 = Wc2.astype(np.float64) @ We.astype(np.float64) @ Wd.astype(np.float64)
    btail = (bc2.astype(np.float64) @ We.astype(np.float64) @ Wd.astype(np.float64)
             + be.astype(np.float64) @ Wd.astype(np.float64) + bd.astype(np.float64))

    xT = c(x.reshape(NT, NB, KC, 128).transpose(0, 3, 2, 1))      # [n,kk,kc,b']
    w1 = c(W1p.reshape(EPAD, KC, 128, HC, 128)
           .transpose(0, 2, 1, 3, 4).reshape(EPAD, 128, KC * HC, 128))
    w2 = W2p.reshape(EPAD, HC, 128, DOUT).transpose(0, 2, 1, 3)
    wc1 = Wc1p.reshape(EPAD, 128, FC, 128)
    wB = c(np.concatenate([w2, wc1], axis=2))                     # [e,*,6,128]
    wth = c(Wtail64.astype(f).reshape(FC, 128, DOUT).transpose(1, 0, 2))
    b1h = b1p.reshape(EPAD, HC, 128).transpose(2, 0, 1).reshape(128, EPAD * HC)

    in_maps = []
    for core in range(NCORES):
        es = slice(core * EPC, (core + 1) * EPC)
        sm = np.empty((128, EPC * HC + FC + 1), f)
        sm[:, :EPC * HC] = b1h[:, core * EPC * HC:(core + 1) * EPC * HC]
        sm[:, EPC * HC:EPC * HC + FC] = bc1_eff.reshape(FC, 128).T
        sm[:, EPC * HC + FC] = btail.astype(f)
        in_maps.append({
            "xT": xT, "wt": wth, "smalls": c(sm),
            "w1": c(w1[es]), "wB": c(wB[es]),
        })
    return in_maps


def kernel(x, W1, b1, W2, b2, Wc1, bc1, Wc2, bc2, We, be, Wd, bd,
           _trace=False):
    if "nc" not in _CACHE:
        _CACHE["nc"] = _build()
    nc = _CACHE["nc"]
    in_maps = _prep(x, W1, b1, W2, b2, Wc1, bc1, Wc2, bc2, We, be, Wd, bd)
    res = run_bass_kernel_spmd(nc, in_maps, list(range(NCORES)), trace=_trace)
    if _trace:
        _CACHE["last_result"] = res
    outT = res.results[0]["outT"]
    return np.ascontiguousarray(outT.T)
